# revision 1
# baseline (speedup 1.0000x reference)
"""Causal self-attention kernel for 8 Trainium2 NeuronCores.

Problem: B=4, T=2048, C=1024, H=16 heads, D=64 (fp32).
  qkv = x @ w_qkv + b_qkv ; causal softmax attention ; y @ w_proj + b_proj

Sharding: DP over batch (4) x TP over heads (2) = 8 cores.
Core c handles batch b=c//2 and heads h0=(c%2)*8 .. h0+7.
Each core computes a partial projection output (its 8 heads' contribution);
the host sums the two TP partials per batch and adds b_proj.

Device-side dataflow per core (all matmuls in float32r — fp32 bits, TF32-like
PE mode at 4x fp32 speed):
  phase 1 (per 512-token chunk n):
    qT/kT [feat, tok] = W_qk^T x^T   (lhsT=W_qk tiles, rhs=x^T chunk)
    v     [tok, feat] = x^T^T W_v    (lhsT=x^T sub-tiles, rhs=W_v)
  attention (per q-group qg=n, per head):
    ST[k,q] = k^T_tile^T @ q^T_chunk   (64-partition quadrant matmuls; k-tile
      pairs share a 2-bank PSUM tile so one exp covers up to 1024 columns;
      diagonal tiles only compute the causally-needed q columns)
    PT = exp(ST - 4.0); diagonal pairs multiplied by precomputed 0/1 masks
    oT_aug[65,q] += v_aug[k,65]^T @ PT  (ones column gives softmax denom row)
    oT[64,q] = oT_aug[0:64] * (1/denom)  (partition-broadcast reciprocal)
  proj (per q-group): out[tok, C] += oT_tiles^T @ w_proj_tiles
"""

import numpy as np

B, T, C = 4, 2048, 1024
H, D = 16, 64
NCORES = 8
HC = H // 2  # heads per core (TP=2)
CEXP = 4.0  # constant softmax offset (scores are in [-4, 4] for this problem)

TN = 512  # token chunk
NCHUNK = T // TN  # 4
KT_C = C // 128  # 8 contraction tiles for C
NQKM = C // 128  # 8 m-tiles for the qk matmul output (1024 feats)
NVSUB = TN // 128  # 4 v sub-tiles per chunk
NKT = T // 128  # 16 k-token tiles
KT_P = (HC * D) // 128  # 4 contraction tiles for proj (512 feats)

_CACHE = {}
_V2 = False  # sequential q/k order + sync DMA queue measured faster on HW


def _build_program(reps=1):
    # reps>1 repeats the whole kernel body inside one program (timing only:
    # the slope between rep counts isolates HW exec time from RPC overhead).
    import concourse.mybir as mybir
    import concourse.tile as tile
    from concourse import bacc

    f32 = mybir.dt.float32
    f32r = mybir.dt.float32r

    nc = bacc.Bacc("TRN2", target_bir_lowering=False, debug=False)

    xT = nc.dram_tensor("xT", [C, T], f32, kind="ExternalInput").ap()
    wqk = nc.dram_tensor("wqk", [C, 2 * HC * D], f32, kind="ExternalInput").ap()
    wv = nc.dram_tensor("wv", [C, HC * D], f32, kind="ExternalInput").ap()
    wproj = nc.dram_tensor("wproj", [HC * D, C], f32, kind="ExternalInput").ap()
    bqk = nc.dram_tensor("bqk", [2 * HC * D], f32, kind="ExternalInput").ap()
    bv = nc.dram_tensor("bv", [HC * D], f32, kind="ExternalInput").ap()
    out = nc.dram_tensor("out", [T, C], f32, kind="ExternalOutput").ap()

    xT_r = xT.rearrange("(ko p) t -> p ko t", p=128)  # [128, 8, 2048]
    wqk_r = wqk.rearrange("(ko p) f -> p ko f", p=128)  # [128, 8, 1024]
    wv_r = wv.rearrange("(ko p) f -> p ko f", p=128)  # [128, 8, 512]
    wproj_r = wproj.rearrange("(ko p) f -> p ko f", p=128)  # [128, 4, 1024]
    bqk_r = bqk.rearrange("(m p) -> p m", p=128)  # [128, 8]

    Exp = mybir.ActivationFunctionType.Exp
    Identity = mybir.ActivationFunctionType.Identity
    dma2 = nc.scalar if _V2 else nc.sync  # secondary DMA issue queue

    with tile.TileContext(nc) as tc:
        with (
            tc.tile_pool(name="pers", bufs=1) as pers,
            tc.tile_pool(name="xc", bufs=2) as xcp,
            tc.tile_pool(name="wqkm", bufs=3) as wqkp,
            tc.tile_pool(name="qtc", bufs=2) as qtcp,
            tc.tile_pool(name="ptp", bufs=4) as ptp,
            tc.tile_pool(name="otc", bufs=2) as otcp,
            tc.tile_pool(name="outp", bufs=2) as outp,
            tc.tile_pool(name="rcp", bufs=1) as rcp,
            tc.tile_pool(name="rcbp", bufs=1) as rcbp,
            tc.tile_pool(name="ps_qv", bufs=2, space="PSUM") as ps_qv,
            tc.tile_pool(name="ps_st", bufs=2, space="PSUM") as ps_st,
            tc.tile_pool(name="ps_ot", bufs=2, space="PSUM") as ps_ot,
        ):
            # --- persistent tiles ---
            kT_sb = pers.tile([128, HC * D // 128, T], f32r)  # [128, 4, 2048]
            v_aug = pers.tile([128, NKT, HC, D + 1], f32r)  # [128,16,8,65]
            wv_sb = pers.tile([128, KT_C, HC * D], f32r)  # [128, 8, 512]
            wpj_sb = pers.tile([128, KT_P, C], f32r)  # [128, 4, 1024]
            bqk_sb = pers.tile([128, NQKM], f32)  # [128, 8]
            bv_bc = pers.tile([128, HC * D], f32)  # [128, 512]
            neg_c = pers.tile([128, 1], f32)
            bv_row = pers.tile([1, HC * D], f32)
            # combined causal masks for diagonal ST pairs: mask_a for the
            # (j0 w=512 | j1 w=384) pair, mask_b for (j2 w=256 | j3 w=256,
            # j3 widened past its causal need to dodge the f32r N<256 penalty).
            mask_a = pers.tile([128, TN + 384], mybir.dt.bfloat16)
            mask_b = pers.tile([128, 256 + 256], mybir.dt.bfloat16)

            nc.vector.memset(v_aug[:].bitcast(f32), 1.0)
            nc.vector.memset(neg_c[:], -CEXP)
            nc.vector.memset(mask_a[:], 1.0)
            nc.vector.memset(mask_b[:], 1.0)
            # each region keeps q' >= kr + excess, where excess is how much
            # the tile was widened past its causal need (128 for the j3 tile)
            for mk, regions in (
                (mask_a, ((0, TN, 0), (TN, 384, 0))),
                (mask_b, ((0, 256, 0), (256, 256, -128))),
            ):
                for off, w, base in regions:
                    nc.gpsimd.affine_select(
                        out=mk[:, off : off + w],
                        in_=mk[:, off : off + w],
                        compare_op=mybir.AluOpType.is_ge,
                        fill=0.0,
                        base=base,
                        pattern=[[1, w]],
                        channel_multiplier=-1,
                    )
            dma2.dma_start(bqk_sb[:], bqk_r)
            dma2.dma_start(bv_row[:], bv[None, :])
            nc.gpsimd.partition_broadcast(bv_bc[:], bv_row[:])
            bv_hd = bv_bc[:].rearrange("p (h d) -> p h d", d=D)  # [128, 8, 64]

            for _rep in range(reps):
                for n in range(NCHUNK):
                    # ---------- phase 1: qk + v for token chunk n ----------
                    xc = xcp.tile([128, KT_C, TN], f32r)
                    _morder = (0, 4, 1, 5, 2, 6, 3, 7)

                    def load_wm(m):
                        wm = wqkp.tile([128, KT_C, 128], f32r, tag="wm")
                        for kh in range(2):  # halves so the kt=0 matmul starts early
                            sl = slice(kh * (KT_C // 2), (kh + 1) * (KT_C // 2))
                            nc.sync.dma_start(
                                wm[:, sl, :],
                                wqk_r[:, sl, m * 128 : (m + 1) * 128].bitcast(f32r),
                            )
                        return wm

                    # first m-tile's weights BEFORE the xc loads: the first qk
                    # matmul needs wm(m0) half 0 + xc kt0, not the whole chunk
                    wm_next = load_wm(_morder[0])
                    for kt in range(KT_C):  # split per k-tile so matmuls start early
                        nc.sync.dma_start(
                            xc[:, kt, :],
                            xT_r[:, kt, n * TN : (n + 1) * TN].bitcast(f32r),
                        )

                    if n == 0:  # needed by the interleaved v matmuls below
                        dma2.dma_start(wv_sb[:], wv_r.bitcast(f32r))
                    qTc = qtcp.tile([128, HC * D // 128, TN], f32r)  # [128, 4, 512]
                    # q/k interleaved so heads unblock early (head pair hh needs
                    # only m=hh and m=4+hh)
                    for step, m in enumerate(_morder):
                        wm = wm_next
                        if step + 1 < len(_morder):
                            wm_next = load_wm(_morder[step + 1])
                        ps = ps_qv.tile([128, TN], f32, tag="qv")
                        for kt in range(KT_C):
                            nc.tensor.matmul(
                                ps[:],
                                wm[:, kt, :],
                                xc[:, kt, :],
                                start=(kt == 0),
                                stop=(kt == KT_C - 1),
                            )
                        if m < 4:  # q features -> per-chunk qT buffer
                            dst = qTc[:, m, :]
                        else:  # k features -> persistent kT
                            dst = kT_sb[:, m - 4, n * TN : (n + 1) * TN]
                        # Identity+bias on ACT (idle during phase 1), except
                        # the last two groups: their slots gate the v matmuls,
                        # and ACT's FIFO may be draining attention exps then
                        if step < 6:
                            nc.scalar.activation(
                                dst, ps[:], Identity, bias=bqk_sb[:, m : m + 1]
                            )
                        else:
                            nc.vector.tensor_scalar_add(
                                dst, ps[:], bqk_sb[:, m : m + 1]
                            )

                        _do_v = (step % 2 == 1) if _V2 else (step == 7)
                        for mm in ([step // 2] if _V2 else [0, 1, 2, 3]) if _do_v else []:
                            ktg = n * NVSUB + mm
                            psv = ps_qv.tile([128, HC * D], f32, tag="qv")
                            for kt in range(KT_C):
                                nc.tensor.matmul(
                                    psv[:],
                                    xc[:, kt, mm * 128 : (mm + 1) * 128],
                                    wv_sb[:, kt, :],
                                    start=(kt == 0),
                                    stop=(kt == KT_C - 1),
                                )
                            nc.vector.tensor_add(
                                out=v_aug[:, ktg, :, 0:D],
                                in0=psv[:].rearrange("p (h d) -> p h d", d=D),
                                in1=bv_hd,
                            )

                    # ---------- attention for q-group qg = n ----------
                    # k-tiles are processed in pairs sharing a 2-bank PSUM tile so
                    # one exp covers up to 1024 columns (amortizes ACT overhead).
                    # Diagonal tiles (kt >= 4*qg) only compute q-columns >= their
                    # k range (causal col-trim): tile j covers q cols 128j..512.
                    qg = n
                    kt_max = NVSUB * (qg + 1)

                    def st_width(kt):
                        j = kt - NVSUB * qg
                        # never narrower than 256: f32r matmuls with free dim
                        # <256 run at 4 cyc/row, so a 128-wide tile costs MORE
                        # than a masked 256-wide one
                        return TN if j < 0 else max(TN - 128 * j, 256)

                    otc = otcp.tile([128, KT_P, TN], f32r)  # [128, 4, 512]
                    for h in range(HC):
                        pb = (h % 2) * 64
                        ko = h // 2
                        pso = ps_ot.tile([D + 1, TN], f32)
                        for ka in range(0, kt_max, 2):
                            kb = ka + 1
                            wa, wb = st_width(ka), st_width(kb)
                            pss = ps_st.tile([128, 2 * TN], f32, tag="pss")
                            for kt, off, w in ((ka, 0, wa), (kb, wa, wb)):
                                nc.tensor.matmul(
                                    pss[:, off : off + w],
                                    kT_sb[pb : pb + 64, ko, kt * 128 : (kt + 1) * 128],
                                    qTc[pb : pb + 64, ko, TN - w : TN],
                                    start=True,
                                    stop=True,
                                )
                            pt = ptp.tile([128, 2 * TN], f32r)
                            nc.scalar.activation(
                                pt[:, 0 : wa + wb], pss[:, 0 : wa + wb], Exp, bias=neg_c[:]
                            )
                            if ka >= NVSUB * qg:  # diagonal pair: one combined mask mul
                                mk = mask_a if wa == TN else mask_b
                                nc.vector.tensor_mul(
                                    out=pt[:, 0 : wa + wb],
                                    in0=pt[:, 0 : wa + wb],
                                    in1=mk[:, 0 : wa + wb],
                                )
                            for kt, off, w in ((ka, 0, wa), (kb, wa, wb)):
                                nc.tensor.matmul(
                                    pso[:, TN - w : TN],
                                    v_aug[:, kt, h, :],
                                    pt[:, off : off + w],
                                    start=(kt == 0),
                                    stop=(kt == kt_max - 1),
                                )
                        rc = rcp.tile([1, TN], f32)
                        nc.vector.reciprocal(rc[:], pso[D : D + 1, :])
                        rcb = rcbp.tile([64, TN], f32)
                        nc.gpsimd.partition_broadcast(rcb[:], rc[:])
                        nc.vector.tensor_mul(
                            out=otc[pb : pb + 64, ko, :], in0=pso[0:D, :], in1=rcb[:]
                        )

                    # ---------- proj for q-group qg ----------
                    # pp shares the ST pool's 2x[128,1024] slots (tag "st"):
                    # attention(qg) is done and attention(qg+1) starts only after
                    # phase 1 of chunk n+1, so the slots are free here.
                    if n == 0:
                        dma2.dma_start(wpj_sb[:], wproj_r.bitcast(f32r))
                    for mm in range(NVSUB):
                        pp = ps_st.tile([128, 2 * TN], f32, tag="pss")
                        for nn in range(2):
                            for kt in range(KT_P):
                                nc.tensor.matmul(
                                    pp[:, nn * TN : (nn + 1) * TN],
                                    otc[:, kt, mm * 128 : (mm + 1) * 128],
                                    wpj_sb[:, kt, nn * TN : (nn + 1) * TN],
                                    start=(kt == 0),
                                    stop=(kt == KT_P - 1),
                                )
                        ob = outp.tile([128, 2 * TN], f32)
                        nc.vector.tensor_copy(ob[:, 0:TN], pp[:, 0:TN])
                        nc.scalar.copy(ob[:, TN:], pp[:, TN:])
                        dma2.dma_start(
                            out[qg * TN + mm * 128 : qg * TN + (mm + 1) * 128, :],
                            ob[:],
                        )

    nc.compile()
    return nc


def _prep_inputs(x, w_qkv, b_qkv, w_proj):
    """Shard full inputs into 8 per-core input maps."""
    x = np.asarray(x, dtype=np.float32)
    w_qkv = np.asarray(w_qkv, dtype=np.float32)
    b_qkv = np.asarray(b_qkv, dtype=np.float32)
    w_proj = np.asarray(w_proj, dtype=np.float32)

    Wq, Wk, Wv = w_qkv[:, :C], w_qkv[:, C : 2 * C], w_qkv[:, 2 * C :]
    bq, bk, bvv = b_qkv[:C], b_qkv[C : 2 * C], b_qkv[2 * C :]
    scale = 1.0 / np.sqrt(np.float32(D))  # 0.125, exact

    in_maps = []
    for c in range(NCORES):
        b, t = divmod(c, 2)
        sl = slice(t * HC * D, (t + 1) * HC * D)
        in_maps.append(
            {
                "xT": np.ascontiguousarray(x[b].T),
                "wqk": np.ascontiguousarray(
                    np.concatenate([Wq[:, sl] * scale, Wk[:, sl]], axis=1)
                ),
                "wv": np.ascontiguousarray(Wv[:, sl]),
                "wproj": np.ascontiguousarray(w_proj[sl, :]),
                "bqk": np.ascontiguousarray(
                    np.concatenate([bq[sl] * scale, bk[sl]])
                ),
                "bv": np.ascontiguousarray(bvv[sl]),
            }
        )
    return in_maps


def _run(x, w_qkv, b_qkv, w_proj, b_proj, trace=False, **trace_kwargs):
    from concourse.bass_utils import run_bass_kernel_spmd

    if "nc" not in _CACHE:
        _CACHE["nc"] = _build_program()
    nc = _CACHE["nc"]

    in_maps = _prep_inputs(x, w_qkv, b_qkv, w_proj)
    res = run_bass_kernel_spmd(
        nc, in_maps, list(range(NCORES)), trace=trace, **trace_kwargs
    )

    b_proj = np.asarray(b_proj, dtype=np.float32)
    y = np.empty((B, T, C), dtype=np.float32)
    for b in range(B):
        y[b] = res.results[2 * b]["out"] + res.results[2 * b + 1]["out"] + b_proj
    return y, res


def kernel(x, w_qkv, b_qkv, w_proj, b_proj):
    y, _ = _run(x, w_qkv, b_qkv, w_proj, b_proj, trace=False)
    return y



# revision 4
# speedup vs baseline: 1.8516x; 1.8516x over previous
"""Causal self-attention kernel for 8 Trainium2 NeuronCores.

Problem: B=4, T=2048, C=1024, H=16 heads, D=64 (fp32).
  qkv = x @ w_qkv + b_qkv ; causal softmax attention ; y @ w_proj + b_proj

Sharding: DP over batch (4) x TP over heads (2) = 8 cores.
Core c handles batch b=c//2 and heads h0=(c%2)*8 .. h0+7.
Each core computes a partial projection output (its 8 heads' contribution);
the host sums the two TP partials per batch and adds b_proj.

Device-side dataflow per core (all matmuls in float32r — fp32 bits, TF32-like
PE mode at 4x fp32 speed):
  phase 1 (per 512-token chunk n):
    qT/kT [feat, tok] = W_qk^T x^T   (lhsT=W_qk tiles, rhs=x^T chunk)
    v     [tok, feat] = x^T^T W_v    (lhsT=x^T sub-tiles, rhs=W_v)
  attention (per q-group qg=n, per head):
    ST[k,q] = k^T_tile^T @ q^T_chunk   (64-partition quadrant matmuls; k-tile
      pairs share a 2-bank PSUM tile so one exp covers up to 1024 columns;
      diagonal tiles only compute the causally-needed q columns)
    PT = exp(ST - 4.0); diagonal pairs multiplied by precomputed 0/1 masks
    oT_aug[65,q] += v_aug[k,65]^T @ PT  (ones column gives softmax denom row)
    oT[64,q] = oT_aug[0:64] * (1/denom)  (partition-broadcast reciprocal)
  proj (per q-group): out[tok, C] += oT_tiles^T @ w_proj_tiles
"""

import numpy as np

B, T, C = 4, 2048, 1024
H, D = 16, 64
NCORES = 8
HC = H // 2  # heads per core (TP=2)
CEXP = 4.0  # constant softmax offset (scores are in [-4, 4] for this problem)

TN = 512  # token chunk
NCHUNK = T // TN  # 4
KT_C = C // 128  # 8 contraction tiles for C
NQKM = C // 128  # 8 m-tiles for the qk matmul output (1024 feats)
NVSUB = TN // 128  # 4 v sub-tiles per chunk
NKT = T // 128  # 16 k-token tiles
KT_P = (HC * D) // 128  # 4 contraction tiles for proj (512 feats)

_CACHE = {}
_V2 = False  # sequential q/k order + sync DMA queue measured faster on HW


def _build_program(reps=1):
    # reps>1 repeats the whole kernel body inside one program (timing only:
    # the slope between rep counts isolates HW exec time from RPC overhead).
    import concourse.mybir as mybir
    import concourse.tile as tile
    from concourse import bacc

    f32 = mybir.dt.float32
    f32r = mybir.dt.float32r

    nc = bacc.Bacc("TRN2", target_bir_lowering=False, debug=False)

    xT = nc.dram_tensor("xT", [C, T], f32, kind="ExternalInput").ap()
    wqk = nc.dram_tensor("wqk", [C, 2 * HC * D], f32, kind="ExternalInput").ap()
    wv = nc.dram_tensor("wv", [C, HC * D], f32, kind="ExternalInput").ap()
    wproj = nc.dram_tensor("wproj", [HC * D, C], f32, kind="ExternalInput").ap()
    bqk = nc.dram_tensor("bqk", [2 * HC * D], f32, kind="ExternalInput").ap()
    bv = nc.dram_tensor("bv", [HC * D], f32, kind="ExternalInput").ap()
    out = nc.dram_tensor("out", [T, C], f32, kind="ExternalOutput").ap()

    xT_r = xT.rearrange("(ko p) t -> p ko t", p=128)  # [128, 8, 2048]
    wqk_r = wqk.rearrange("(ko p) f -> p ko f", p=128)  # [128, 8, 1024]
    wv_r = wv.rearrange("(ko p) f -> p ko f", p=128)  # [128, 8, 512]
    wproj_r = wproj.rearrange("(ko p) f -> p ko f", p=128)  # [128, 4, 1024]
    bqk_r = bqk.rearrange("(m p) -> p m", p=128)  # [128, 8]

    Exp = mybir.ActivationFunctionType.Exp
    Identity = mybir.ActivationFunctionType.Identity
    dma2 = nc.scalar if _V2 else nc.sync  # secondary DMA issue queue

    with tile.TileContext(nc) as tc:
        with (
            tc.tile_pool(name="pers", bufs=1) as pers,
            tc.tile_pool(name="xc", bufs=2) as xcp,
            tc.tile_pool(name="wqkm", bufs=3) as wqkp,
            tc.tile_pool(name="qtc", bufs=2) as qtcp,
            tc.tile_pool(name="ptp", bufs=4) as ptp,
            tc.tile_pool(name="otc", bufs=2) as otcp,
            tc.tile_pool(name="outp", bufs=2) as outp,
            tc.tile_pool(name="rcp", bufs=1) as rcp,
            tc.tile_pool(name="rcbp", bufs=1) as rcbp,
            tc.tile_pool(name="ps_qv", bufs=2, space="PSUM") as ps_qv,
            tc.tile_pool(name="ps_st", bufs=2, space="PSUM") as ps_st,
            tc.tile_pool(name="ps_ot", bufs=2, space="PSUM") as ps_ot,
        ):
            # --- persistent tiles ---
            kT_sb = pers.tile([128, HC * D // 128, T], f32r)  # [128, 4, 2048]
            v_aug = pers.tile([128, NKT, HC, D + 1], f32r)  # [128,16,8,65]
            wv_sb = pers.tile([128, KT_C, HC * D], f32r)  # [128, 8, 512]
            wpj_sb = pers.tile([128, KT_P, C], f32r)  # [128, 4, 1024]
            bqk_sb = pers.tile([128, NQKM], f32)  # [128, 8]
            bv_bc = pers.tile([128, HC * D], f32)  # [128, 512]
            neg_c = pers.tile([128, 1], f32)
            bv_row = pers.tile([1, HC * D], f32)
            # combined causal masks for diagonal ST pairs: mask_a for the
            # (j0 w=512 | j1 w=384) pair, mask_b for (j2 w=256 | j3 w=256,
            # j3 widened past its causal need to dodge the f32r N<256 penalty).
            mask_a = pers.tile([128, TN + 384], mybir.dt.bfloat16)
            mask_b = pers.tile([128, 256 + 256], mybir.dt.bfloat16)

            # only the ones-column needs initializing; data cols are written
            # by the v folds (full-tile memset cost ~10us DVE at startup)
            nc.vector.memset(v_aug[:, :, :, D : D + 1].bitcast(f32), 1.0)
            nc.vector.memset(neg_c[:], -CEXP)
            nc.vector.memset(mask_a[:], 1.0)
            nc.vector.memset(mask_b[:], 1.0)
            # each region keeps q' >= kr + excess, where excess is how much
            # the tile was widened past its causal need (128 for the j3 tile)
            for mk, regions in (
                (mask_a, ((0, TN, 0), (TN, 384, 0))),
                (mask_b, ((0, 256, 0), (256, 256, -128))),
            ):
                for off, w, base in regions:
                    nc.gpsimd.affine_select(
                        out=mk[:, off : off + w],
                        in_=mk[:, off : off + w],
                        compare_op=mybir.AluOpType.is_ge,
                        fill=0.0,
                        base=base,
                        pattern=[[1, w]],
                        channel_multiplier=-1,
                    )
            dma2.dma_start(bqk_sb[:], bqk_r)
            dma2.dma_start(bv_row[:], bv[None, :])
            nc.gpsimd.partition_broadcast(bv_bc[:], bv_row[:])
            bv_hd = bv_bc[:].rearrange("p (h d) -> p h d", d=D)  # [128, 8, 64]

            for _rep in range(reps):
                for n in range(NCHUNK):
                    # ---------- phase 1: qk + v for token chunk n ----------
                    xc = xcp.tile([128, KT_C, TN], f32r)
                    _morder = (0, 4, 1, 5, 2, 6, 3, 7)

                    def load_wm(m):
                        wm = wqkp.tile([128, KT_C, 128], f32r, tag="wm")
                        for kh in range(2):  # halves so the kt=0 matmul starts early
                            sl = slice(kh * (KT_C // 2), (kh + 1) * (KT_C // 2))
                            nc.sync.dma_start(
                                wm[:, sl, :],
                                wqk_r[:, sl, m * 128 : (m + 1) * 128].bitcast(f32r),
                            )
                        return wm

                    # first m-tile's weights BEFORE the xc loads: the first qk
                    # matmul needs wm(m0) half 0 + xc kt0, not the whole chunk
                    wm_next = load_wm(_morder[0])
                    for kt in range(KT_C):  # split per k-tile so matmuls start early
                        # alternate issue queues: halves the serial DMA chain
                        # ahead of the first matmul of each chunk
                        q = nc.sync if kt % 2 == 0 else nc.scalar
                        q.dma_start(
                            xc[:, kt, :],
                            xT_r[:, kt, n * TN : (n + 1) * TN].bitcast(f32r),
                        )

                    if n == 0:  # needed by the interleaved v matmuls below
                        dma2.dma_start(wv_sb[:], wv_r.bitcast(f32r))
                    qTc = qtcp.tile([128, HC * D // 128, TN], f32r)  # [128, 4, 512]
                    # q/k interleaved so heads unblock early (head pair hh needs
                    # only m=hh and m=4+hh)
                    for step, m in enumerate(_morder):
                        wm = wm_next
                        if step + 1 < len(_morder):
                            wm_next = load_wm(_morder[step + 1])
                        ps = ps_qv.tile([128, TN], f32, tag="qv")
                        for kt in range(KT_C):
                            nc.tensor.matmul(
                                ps[:],
                                wm[:, kt, :],
                                xc[:, kt, :],
                                start=(kt == 0),
                                stop=(kt == KT_C - 1),
                            )
                        if m < 4:  # q features -> per-chunk qT buffer
                            dst = qTc[:, m, :]
                        else:  # k features -> persistent kT
                            dst = kT_sb[:, m - 4, n * TN : (n + 1) * TN]
                        # Identity+bias on ACT (idle during phase 1), except
                        # the last two groups: their slots gate the v matmuls,
                        # and ACT's FIFO may be draining attention exps then
                        if step < 6:
                            nc.scalar.activation(
                                dst, ps[:], Identity, bias=bqk_sb[:, m : m + 1]
                            )
                        else:
                            nc.vector.tensor_scalar_add(
                                dst, ps[:], bqk_sb[:, m : m + 1]
                            )

                        _do_v = (step % 2 == 1) if _V2 else (step == 7)
                        for mm in ([step // 2] if _V2 else [0, 1, 2, 3]) if _do_v else []:
                            ktg = n * NVSUB + mm
                            psv = ps_qv.tile([128, HC * D], f32, tag="qv")
                            for kt in range(KT_C):
                                nc.tensor.matmul(
                                    psv[:],
                                    xc[:, kt, mm * 128 : (mm + 1) * 128],
                                    wv_sb[:, kt, :],
                                    start=(kt == 0),
                                    stop=(kt == KT_C - 1),
                                )
                            nc.vector.tensor_add(
                                out=v_aug[:, ktg, :, 0:D],
                                in0=psv[:].rearrange("p (h d) -> p h d", d=D),
                                in1=bv_hd,
                            )

                    # ---------- attention for q-group qg = n ----------
                    # k-tiles are processed in pairs sharing a 2-bank PSUM tile so
                    # one exp covers up to 1024 columns (amortizes ACT overhead).
                    # Diagonal tiles (kt >= 4*qg) only compute q-columns >= their
                    # k range (causal col-trim): tile j covers q cols 128j..512.
                    qg = n
                    kt_max = NVSUB * (qg + 1)

                    def st_width(kt):
                        j = kt - NVSUB * qg
                        # never narrower than 256: f32r matmuls with free dim
                        # <256 run at 4 cyc/row, so a 128-wide tile costs MORE
                        # than a masked 256-wide one
                        return TN if j < 0 else max(TN - 128 * j, 256)

                    otc = otcp.tile([128, KT_P, TN], f32r)  # [128, 4, 512]
                    for h in range(HC):
                        pb = (h % 2) * 64
                        ko = h // 2
                        pso = ps_ot.tile([D + 1, TN], f32)
                        for ka in range(0, kt_max, 2):
                            kb = ka + 1
                            wa, wb = st_width(ka), st_width(kb)
                            pss = ps_st.tile([128, 2 * TN], f32, tag="pss")
                            for kt, off, w in ((ka, 0, wa), (kb, wa, wb)):
                                nc.tensor.matmul(
                                    pss[:, off : off + w],
                                    kT_sb[pb : pb + 64, ko, kt * 128 : (kt + 1) * 128],
                                    qTc[pb : pb + 64, ko, TN - w : TN],
                                    start=True,
                                    stop=True,
                                )
                            pt = ptp.tile([128, 2 * TN], f32r)
                            nc.scalar.activation(
                                pt[:, 0 : wa + wb], pss[:, 0 : wa + wb], Exp, bias=neg_c[:]
                            )
                            if ka >= NVSUB * qg:  # diagonal pair: one combined mask mul
                                mk = mask_a if wa == TN else mask_b
                                nc.vector.tensor_mul(
                                    out=pt[:, 0 : wa + wb],
                                    in0=pt[:, 0 : wa + wb],
                                    in1=mk[:, 0 : wa + wb],
                                )
                            for kt, off, w in ((ka, 0, wa), (kb, wa, wb)):
                                nc.tensor.matmul(
                                    pso[:, TN - w : TN],
                                    v_aug[:, kt, h, :],
                                    pt[:, off : off + w],
                                    start=(kt == 0),
                                    stop=(kt == kt_max - 1),
                                )
                        rc = rcp.tile([1, TN], f32)
                        nc.vector.reciprocal(rc[:], pso[D : D + 1, :])
                        rcb = rcbp.tile([64, TN], f32)
                        nc.gpsimd.partition_broadcast(rcb[:], rc[:])
                        nc.vector.tensor_mul(
                            out=otc[pb : pb + 64, ko, :], in0=pso[0:D, :], in1=rcb[:]
                        )

                    # ---------- proj for q-group qg ----------
                    # pp shares the ST pool's 2x[128,1024] slots (tag "st"):
                    # attention(qg) is done and attention(qg+1) starts only after
                    # phase 1 of chunk n+1, so the slots are free here.
                    if n == 0:
                        dma2.dma_start(wpj_sb[:], wproj_r.bitcast(f32r))
                    for mm in range(NVSUB):
                        pp = ps_st.tile([128, 2 * TN], f32, tag="pss")
                        for nn in range(2):
                            for kt in range(KT_P):
                                nc.tensor.matmul(
                                    pp[:, nn * TN : (nn + 1) * TN],
                                    otc[:, kt, mm * 128 : (mm + 1) * 128],
                                    wpj_sb[:, kt, nn * TN : (nn + 1) * TN],
                                    start=(kt == 0),
                                    stop=(kt == KT_P - 1),
                                )
                        ob = outp.tile([128, 2 * TN], f32)
                        nc.vector.tensor_copy(ob[:, 0:TN], pp[:, 0:TN])
                        nc.scalar.copy(ob[:, TN:], pp[:, TN:])
                        # output DMAs on the scalar queue so they don't delay
                        # the next chunk's weight/activation input loads
                        nc.scalar.dma_start(
                            out[qg * TN + mm * 128 : qg * TN + (mm + 1) * 128, :],
                            ob[:],
                        )

    nc.compile()
    return nc


def _prep_inputs(x, w_qkv, b_qkv, w_proj):
    """Shard full inputs into 8 per-core input maps."""
    x = np.asarray(x, dtype=np.float32)
    w_qkv = np.asarray(w_qkv, dtype=np.float32)
    b_qkv = np.asarray(b_qkv, dtype=np.float32)
    w_proj = np.asarray(w_proj, dtype=np.float32)

    Wq, Wk, Wv = w_qkv[:, :C], w_qkv[:, C : 2 * C], w_qkv[:, 2 * C :]
    bq, bk, bvv = b_qkv[:C], b_qkv[C : 2 * C], b_qkv[2 * C :]
    scale = 1.0 / np.sqrt(np.float32(D))  # 0.125, exact

    in_maps = []
    for c in range(NCORES):
        b, t = divmod(c, 2)
        sl = slice(t * HC * D, (t + 1) * HC * D)
        in_maps.append(
            {
                "xT": np.ascontiguousarray(x[b].T),
                "wqk": np.ascontiguousarray(
                    np.concatenate([Wq[:, sl] * scale, Wk[:, sl]], axis=1)
                ),
                "wv": np.ascontiguousarray(Wv[:, sl]),
                "wproj": np.ascontiguousarray(w_proj[sl, :]),
                "bqk": np.ascontiguousarray(
                    np.concatenate([bq[sl] * scale, bk[sl]])
                ),
                "bv": np.ascontiguousarray(bvv[sl]),
            }
        )
    return in_maps


def _run(x, w_qkv, b_qkv, w_proj, b_proj, trace=False, **trace_kwargs):
    from concourse.bass_utils import run_bass_kernel_spmd

    if "nc" not in _CACHE:
        _CACHE["nc"] = _build_program()
    nc = _CACHE["nc"]

    in_maps = _prep_inputs(x, w_qkv, b_qkv, w_proj)
    res = run_bass_kernel_spmd(
        nc, in_maps, list(range(NCORES)), trace=trace, **trace_kwargs
    )

    b_proj = np.asarray(b_proj, dtype=np.float32)
    y = np.empty((B, T, C), dtype=np.float32)
    for b in range(B):
        y[b] = res.results[2 * b]["out"] + res.results[2 * b + 1]["out"] + b_proj
    return y, res


def kernel(x, w_qkv, b_qkv, w_proj, b_proj):
    y, _ = _run(x, w_qkv, b_qkv, w_proj, b_proj, trace=False)
    return y

